# revision 1
# baseline (speedup 1.0000x reference)
"""Masked causal self-attention (single head) on 8 Trainium2 NeuronCores.

Problem: x[4,4096,1024], mask[4,4096] (key padding), Wq/Wk/Wv[128,1024],
bq/bk/bv[128] -> out[4,4096,128]:
    q = x@Wq.T+bq; k = x@Wk.T+bk; v = x@Wv.T+bv
    out = softmax(causal_mask(q@k.T/sqrt(128)) + key_padding) @ v

Sharding (SPMD, one program on 8 cores): core c = (batch b=c//2, parity
p=c%2). Each core computes K/V for its full batch (replicated within the
pair) and handles the interleaved query 128-row tiles {2*t+p : t in 0..15}
— interleaving balances the causal (triangular) work between the pair.

Device algorithm per core:
  - All matmuls run in float32r (single-pass fp32, 1 cycle/row at moving
    free-dim >= 256, vs 4 cycles/row for exact fp32) via AP bitcasts.
  - Projections are computed transposed ([head, seq] layout) with the
    d-contraction on partitions: K^T/V^T/Q^T = W.T-chunks @ x^T-chunks,
    accumulated in PSUM over 8 d-chunks. Biases are folded in during the
    PSUM->SBUF eviction (per-partition scalar add). The 1/sqrt(128) score
    scale is folded into Wq on the host.
  - V^T is transposed back to [seq,head] tiles with the PE (stationary
    operand of attn@V).
  - Scores are computed transposed: S^T[k,q] = (K^T-tile).T @ Q^T-chunk.
    exp() runs on the scalar engine straight out of PSUM; its per-partition
    bias argument carries the key-padding mask (-1e30 for masked keys).
    Softmax max-subtraction is skipped: scores are ~N(0,1) by construction
    (matches jax softmax mathematically; no overflow in fp32).
  - The causal mask is data-driven so the program is core-uniform: a 0/1
    tile M = (qg >= kg) (qg per-core query indices from DRAM, kg an iota)
    multiplies exp(S^T) for the ~diagonal k-tiles only (into a fresh tile,
    pt2, so every tile has a single writer engine).
  - attn@V accumulates transposed, a whole 512-query chunk at a time:
    outT[h, qchunk] += V_kt.T @ PT_kt and den[*, qchunk] += ones.T @ PT_kt
    (the ones-matmul gives the softmax denominator replicated across
    partitions, so normalization is a plain elementwise multiply).
    The output leaves the device as [H, NQ]; the host transposes.

Hardware instructions carry a single semaphore-wait slot; Bacc.compile()
legalizes multi-wait instructions (generate_event_semaphores).
"""

import sys

sys.path.insert(0, "/opt/trn_rl_repo")

import numpy as np

import concourse.bass as bass
import concourse.bacc as bacc
import concourse.tile as tile
from concourse import mybir
from concourse.masks import make_identity
from concourse import bass_utils

F32 = mybir.dt.float32
F32R = mybir.dt.float32r
B, S, D, H = 4, 4096, 1024, 128
NQ = S // 2          # queries owned per core (2048)
DC = D // 128        # 8 d-chunks
SCH = S // 512       # 8 seq chunks of 512
NKT = S // 128       # 32 key tiles
NEG = -1.0e30


def _build_program():
    nc = bacc.Bacc("TRN2", target_bir_lowering=False)

    xT_d = nc.dram_tensor("xT", [D, S], F32R, kind="ExternalInput")
    xqT_d = nc.dram_tensor("xqT", [D, NQ], F32R, kind="ExternalInput")
    wqT_d = nc.dram_tensor("wqT", [128, DC * H], F32R, kind="ExternalInput")
    wkT_d = nc.dram_tensor("wkT", [128, DC * H], F32R, kind="ExternalInput")
    wvT_d = nc.dram_tensor("wvT", [128, DC * H], F32R, kind="ExternalInput")
    bq_d = nc.dram_tensor("bq", [H, 1], F32, kind="ExternalInput")
    bk_d = nc.dram_tensor("bk", [H, 1], F32, kind="ExternalInput")
    bv_d = nc.dram_tensor("bv", [H, 1], F32, kind="ExternalInput")
    mb_d = nc.dram_tensor("maskbias", [128, NKT], F32, kind="ExternalInput")
    qg_d = nc.dram_tensor("qg", [4, 512], F32, kind="ExternalInput")
    o_d = nc.dram_tensor("o", [H, NQ], F32, kind="ExternalOutput")

    with tile.TileContext(nc) as tc:
        with (
            tc.tile_pool(name="consts", bufs=1) as consts,
            tc.tile_pool(name="big", bufs=1) as big,
            tc.tile_pool(name="vtiles", bufs=NKT) as vtiles,
            tc.tile_pool(name="ptp", bufs=6) as ptp,
            tc.tile_pool(name="pt2p", bufs=4) as pt2p,
        ):
            # ---- constants ----
            ident = consts.tile([128, 128], F32)
            make_identity(nc, ident)
            ones_f = consts.tile([128, 128], F32, tag="ones_f")
            nc.vector.memset(ones_f, 1.0)
            ones = consts.tile([128, 128], F32R)
            nc.vector.tensor_copy(ones, ones_f)
            kg = consts.tile([128, NKT], F32)
            nc.gpsimd.iota(
                kg, pattern=[[128, NKT]], base=0, channel_multiplier=1,
                allow_small_or_imprecise_dtypes=True,
            )
            mb = consts.tile([128, NKT], F32)
            qg_b = []
            for _ in range(4):
                qg_t = consts.tile([128, 512], F32, tag="qg_b")
                qg_b.append(qg_t)

            def load_small_consts():
                nc.sync.dma_start(out=mb, in_=mb_d[:, :])
                for jj in range(4):
                    row = qg_d[jj, :]
                    nc.gpsimd.dma_start(
                        out=qg_b[jj],
                        in_=bass.AP(tensor=row.tensor, offset=row.offset,
                                    ap=[[0, 128]] + list(row.ap)),
                    )
            w_sb = {}
            for name, dram in (("q", wqT_d), ("k", wkT_d), ("v", wvT_d)):
                t = consts.tile([128, DC, H], F32R, tag=f"w_{name}")
                nc.sync.dma_start(out=t, in_=dram[:, :].rearrange("p (c h) -> p c h", c=DC))
                w_sb[name] = t
            b_sb = {}
            for name, dram in (("q", bq_d), ("k", bk_d), ("v", bv_d)):
                t = consts.tile([H, 1], F32, tag=f"b_{name}")
                nc.sync.dma_start(out=t, in_=dram[:, :])
                b_sb[name] = t
            KT = big.tile([128, S], F32R, tag="KT")     # K^T [h, k]
            QT = big.tile([128, NQ], F32R, tag="QT")    # Q^T [h, q]

            # ---- interleaved projections + attention ----
            # attention chunk j only needs k-tiles 0..8j+7 (causal), i.e.
            # K/V from s-chunks 0..2j+1 and Q chunk j: project exactly the
            # two new s-chunks per block, then run the chunk's attention.
            # PE therefore has dense work while the remaining x^T streams in.
            with (
                tc.tile_pool(name="vt_sb", bufs=1) as vt_sb_pool,
                tc.tile_pool(name="xp", bufs=20) as xp,
                tc.tile_pool(name="xqp", bufs=8) as xqp,
                tc.tile_pool(name="kps", bufs=1, space="PSUM") as kps,
                tc.tile_pool(name="vps", bufs=1, space="PSUM") as vps,
                tc.tile_pool(name="qps", bufs=1, space="PSUM") as qps,
                tc.tile_pool(name="tps", bufs=1, space="PSUM") as tps,
                tc.tile_pool(name="sp", bufs=2, space="PSUM") as sp,
                tc.tile_pool(name="op", bufs=1, space="PSUM") as op,
                tc.tile_pool(name="dp", bufs=1, space="PSUM") as dp,
                tc.tile_pool(name="osb", bufs=2) as osb,
                tc.tile_pool(name="rp", bufs=2) as rp,
            ):
                VT = vt_sb_pool.tile([128, S], F32, tag="VT")
                v_t = [None] * NKT

                def project_sc(sc):
                    kpsum = kps.tile([128, 512], F32)
                    vpsum = vps.tile([128, 512], F32)
                    for dc in range(DC):
                        xt = xp.tile([128, 512], F32R, tag="xt")
                        nc.sync.dma_start(
                            out=xt,
                            in_=xT_d[dc * 128:(dc + 1) * 128, sc * 512:(sc + 1) * 512],
                        )
                        nc.tensor.matmul(kpsum, w_sb["k"][:, dc, :], xt,
                                         start=(dc == 0), stop=(dc == DC - 1))
                        nc.tensor.matmul(vpsum, w_sb["v"][:, dc, :], xt,
                                         start=(dc == 0), stop=(dc == DC - 1))
                    nc.vector.tensor_scalar_add(
                        KT[:, sc * 512:(sc + 1) * 512], kpsum, b_sb["k"])
                    nc.vector.tensor_scalar_add(
                        VT[:, sc * 512:(sc + 1) * 512], vpsum, b_sb["v"])
                    for kt in range(4 * sc, 4 * sc + 4):
                        tpsum = tps.tile([128, 128], F32)
                        nc.tensor.transpose(
                            tpsum, VT[:, kt * 128:(kt + 1) * 128], ident)
                        vt = vtiles.tile([128, H], F32R, tag="v_t")
                        nc.vector.tensor_copy(vt, tpsum)
                        v_t[kt] = vt

                def project_q(jc):
                    qpsum = qps.tile([128, 512], F32)
                    for dc in range(DC):
                        xqt = xqp.tile([128, 512], F32R, tag="xqt")
                        nc.sync.dma_start(
                            out=xqt,
                            in_=xqT_d[dc * 128:(dc + 1) * 128, jc * 512:(jc + 1) * 512],
                        )
                        nc.tensor.matmul(qpsum, w_sb["q"][:, dc, :], xqt,
                                         start=(dc == 0), stop=(dc == DC - 1))
                    nc.vector.tensor_scalar_add(
                        QT[:, jc * 512:(jc + 1) * 512], qpsum, b_sb["q"])

                for j in range(4):
                    project_sc(2 * j)
                    if j == 0:
                        load_small_consts()
                    project_q(j)
                    project_sc(2 * j + 1)

                    n_kt = 8 * j + 8
                    outp = op.tile([128, 512], F32)
                    denp = dp.tile([128, 512], F32)
                    pts = [None] * n_kt

                    def score_exp(kt, j=j, pts=pts):
                        spsum = sp.tile([128, 512], F32)
                        nc.tensor.matmul(
                            spsum, KT[:, kt * 128:(kt + 1) * 128],
                            QT[:, j * 512:(j + 1) * 512], start=True, stop=True)
                        pt = ptp.tile([128, 512], F32R, tag="pt")
                        nc.scalar.activation(
                            pt, spsum, mybir.ActivationFunctionType.Exp,
                            bias=mb[:, kt:kt + 1], scale=1.0)
                        if kt >= 8 * j:
                            pt2 = pt2p.tile([128, 512], F32R, tag="pt2")
                            nc.vector.scalar_tensor_tensor(
                                pt2, qg_b[j], kg[:, kt:kt + 1], pt,
                                mybir.AluOpType.is_ge, mybir.AluOpType.mult)
                            pts[kt] = pt2
                        else:
                            pts[kt] = pt

                    def pv(kt, j=j, pts=pts, outp=outp, denp=denp, n_kt=n_kt):
                        nc.tensor.matmul(outp, v_t[kt], pts[kt],
                                         start=(kt == 0), stop=(kt == n_kt - 1))
                        nc.tensor.matmul(denp, ones, pts[kt],
                                         start=(kt == 0), stop=(kt == n_kt - 1))

                    # software-pipelined: PE does scores(kt+1) while the
                    # scalar engine exps scores(kt); PV lags one step
                    score_exp(0)
                    for kt in range(1, n_kt):
                        score_exp(kt)
                        pv(kt - 1)
                    pv(n_kt - 1)

                    r_t = rp.tile([128, 512], F32, tag="r")
                    nc.vector.reciprocal(r_t, denp)
                    o_sb = osb.tile([128, 512], F32, tag="o")
                    nc.vector.tensor_mul(o_sb, outp, r_t)
                    nc.sync.dma_start(
                        out=o_d[:, j * 512:(j + 1) * 512], in_=o_sb)
    nc.compile()
    return nc


def check_matmul_waits(nc, limit=1):
    bad = []
    for f in nc.m.functions:
        for bb in f.blocks:
            for i in bb.instructions:
                if i.opcode == "Matmult":
                    w = i.sync_info.on_wait if i.sync_info else []
                    if len(w) > limit:
                        bad.append((i.name, [(x.ant_name, x.wait_value) for x in w]))
    return bad


_NC_CACHE = {}


def _get_program():
    if "nc" not in _NC_CACHE:
        _NC_CACHE["nc"] = _build_program()
    return _NC_CACHE["nc"]


def _make_in_maps(x, mask, Wq, bq, Wk, bk, Wv, bv):
    x = np.asarray(x, np.float32)
    mask = np.asarray(mask)
    scale = 1.0 / np.sqrt(np.float32(H))
    def pack_w(w):
        # [H,D] -> w.T [D,H] -> partition-major [128, DC*H] for a single
        # contiguous-burst DMA into the SBUF weight tile
        wT = np.asarray(w, np.float32).T.reshape(DC, 128, H)
        return np.ascontiguousarray(wT.transpose(1, 0, 2).reshape(128, DC * H))

    wqT = pack_w(np.asarray(Wq, np.float32) * scale)
    wkT = pack_w(Wk)
    wvT = pack_w(Wv)
    bq_c = (np.asarray(bq, np.float32) * scale).reshape(H, 1).copy()
    bk_c = np.asarray(bk, np.float32).reshape(H, 1).copy()
    bv_c = np.asarray(bv, np.float32).reshape(H, 1).copy()

    in_maps = []
    for c in range(8):
        b, p = c // 2, c % 2
        xT = np.ascontiguousarray(x[b].T)                      # [D, S]
        gt = 2 * np.arange(16) + p                             # owned global q-tiles
        cols = (gt[:, None] * 128 + np.arange(128)[None, :]).reshape(-1)
        xqT = np.ascontiguousarray(xT[:, cols])                # [D, NQ]
        mbias = np.where(mask[b] == 0, np.float32(NEG), np.float32(0.0))
        mbias = np.ascontiguousarray(mbias.reshape(NKT, 128).T.astype(np.float32))
        qg = cols.reshape(4, 512).astype(np.float32)
        in_maps.append({
            "xT": xT, "xqT": xqT, "wqT": wqT, "wkT": wkT, "wvT": wvT,
            "bq": bq_c, "bk": bk_c, "bv": bv_c, "maskbias": mbias,
            "qg": np.ascontiguousarray(qg),
        })
    return in_maps


def _install_ntff_hook():
    # the trimmed antenv package lacks axon_hooks; recreate it and wire the
    # ctypes NTFF profiling hook from trn_agent_boot so trace=True works
    import types
    if "antenv.axon_hooks" in sys.modules:
        return
    import antenv
    mod = types.ModuleType("antenv.axon_hooks")
    _hook = [None]
    mod.set_axon_ntff_profile_hook = lambda h: _hook.__setitem__(0, h)
    mod.get_axon_ntff_profile_hook = lambda: _hook[0]
    sys.modules["antenv.axon_hooks"] = mod
    antenv.axon_hooks = mod
    from trn_agent_boot.trn_boot import _ntff_profile_via_ctypes
    mod.set_axon_ntff_profile_hook(
        _ntff_profile_via_ctypes("/opt/axon/libaxon_pjrt.so"))


def run(inputs, trace=False, tmpdir=None):
    if trace:
        try:
            _install_ntff_hook()
        except Exception as e:
            print("ntff hook install failed:", e)
    nc = _get_program()
    in_maps = _make_in_maps(**inputs)
    res = bass_utils.run_bass_kernel_spmd(
        nc, in_maps, core_ids=list(range(8)), trace=trace, tmpdir=tmpdir)
    out = np.empty((B, S, H), np.float32)
    for c in range(8):
        b, p = c // 2, c % 2
        o = res.results[c]["o"]                                # [H, NQ]
        for lt in range(16):
            g = 2 * lt + p
            out[b, g * 128:(g + 1) * 128, :] = o[:, lt * 128:(lt + 1) * 128].T
    return out, res


def kernel(**inputs) -> np.ndarray:
    out, _ = run(inputs, trace=False)
    return out



# revision 3
# speedup vs baseline: 1.3790x; 1.3790x over previous
"""Masked causal self-attention (single head) on 8 Trainium2 NeuronCores.

Problem: x[4,4096,1024], mask[4,4096] (key padding), Wq/Wk/Wv[128,1024],
bq/bk/bv[128] -> out[4,4096,128]:
    q = x@Wq.T+bq; k = x@Wk.T+bk; v = x@Wv.T+bv
    out = softmax(causal_mask(q@k.T/sqrt(128)) + key_padding) @ v

Sharding (SPMD, one program on 8 cores): core c = (batch b=c//2, parity
p=c%2). Each core computes K/V for its full batch and handles the
interleaved query 128-row tiles {2*t+p : t in 0..15} — interleaving
balances the causal (triangular) work between the pair.

To keep the program core-uniform, the host PERMUTES x's sequence tiles per
core so the core's own query tiles always sit at EVEN 128-column positions
(p=0: natural order; p=1: pairwise swap). All causal structure is then
position-uniform; the one residual parity difference (whether the odd
neighbor tile is a past or future key) is a data-driven [128,128] 0/1 mask.

Device algorithm per core (everything on the PE runs in bf16, 1 cycle/row
at any moving size; PSUM accumulates fp32):
  - Projections K^T/V^T/Q^T ([head, seq] layout, d-contraction on
    partitions) accumulate over 8 d-chunks in PSUM; Q uses the same
    resident x tiles (even 128-blocks only => no second x read). Biases
    fold in during PSUM->SBUF eviction; the 1/sqrt(128) score scale is
    folded into Wq/bq on the host.
  - V^T transposes back to [key, head] tiles on the PE; the key-padding
    mask scales V rows during eviction and also forms a 129th "ones"
    column => masked keys drop out of BOTH the numerator and denominator.
  - Scores S^T[k,q] = KT-tile.T @ QT-chunk; exp() on the scalar engine
    straight out of PSUM (softmax max-subtraction skipped: scores are
    ~N(0,1) by construction). Causal edge tiles multiply by a constant
    triangular tile (diag) / the parity mask (odd neighbor).
  - attn@V runs with the exp tile as the STATIONARY operand and
    [V | mask] (129 cols) as the moving operand: out[q,128:129] then IS
    the softmax denominator, accumulated for free, and the output comes
    out in [q, head] orientation. Two accumulation groups share each PSUM
    bank (single start/stop per bank; PSUM pending-zero semantics make
    the second group's first touch an overwrite).
  - Normalization is reciprocal + per-partition scalar multiply.
  - Projections for chunk j+1 are emission-interleaved into attention
    chunk j so the PE has filler while the scalar engine exps.
"""

import sys

sys.path.insert(0, "/opt/trn_rl_repo")

import numpy as np
import ml_dtypes

import concourse.bass as bass
import concourse.bacc as bacc
import concourse.tile as tile
from concourse import mybir
from concourse import bass_utils

F32 = mybir.dt.float32
BF16 = mybir.dt.bfloat16
B, S, D, H = 4, 4096, 1024, 128
NQ = S // 2          # queries owned per core (2048)
DC = D // 128        # 8 d-chunks
NSC = S // 512       # 8 seq chunks of 512
NKT = S // 128       # 32 key tile positions
NJ = 4               # attention chunks of 512 owned queries


def _build_program():
    nc = bacc.Bacc("TRN2", target_bir_lowering=False)

    xT_d = nc.dram_tensor("xT", [D, S], BF16, kind="ExternalInput")
    wqT_d = nc.dram_tensor("wqT", [128, DC * H], BF16, kind="ExternalInput")
    wkT_d = nc.dram_tensor("wkT", [128, DC * H], BF16, kind="ExternalInput")
    wvT_d = nc.dram_tensor("wvT", [128, DC * H], BF16, kind="ExternalInput")
    bq_d = nc.dram_tensor("bq", [H, 1], F32, kind="ExternalInput")
    bk_d = nc.dram_tensor("bk", [H, 1], F32, kind="ExternalInput")
    bv_d = nc.dram_tensor("bv", [H, 1], F32, kind="ExternalInput")
    ident_d = nc.dram_tensor("ident", [128, 128], BF16, kind="ExternalInput")
    tri_d = nc.dram_tensor("tri", [128, 128], BF16, kind="ExternalInput")
    mb_d = nc.dram_tensor("maskB", [128, 128], BF16, kind="ExternalInput")
    m01_d = nc.dram_tensor("m01", [128, NKT], F32, kind="ExternalInput")
    m01b_d = nc.dram_tensor("m01b", [128, NKT], BF16, kind="ExternalInput")
    o_d = nc.dram_tensor("o", [NQ, H], F32, kind="ExternalOutput")

    with tile.TileContext(nc) as tc:
        with (
            tc.tile_pool(name="consts", bufs=1) as consts,
            tc.tile_pool(name="big", bufs=1) as big,
            tc.tile_pool(name="vtiles", bufs=NKT) as vtiles,
            tc.tile_pool(name="xp", bufs=16) as xp,
            tc.tile_pool(name="vstage", bufs=2) as vstage,
            tc.tile_pool(name="ptp", bufs=6) as ptp,
            tc.tile_pool(name="pt2p", bufs=4) as pt2p,
            tc.tile_pool(name="osb", bufs=4) as osb,
            tc.tile_pool(name="rp", bufs=4) as rp,
            tc.tile_pool(name="kvps", bufs=1, space="PSUM") as kvps,
            tc.tile_pool(name="scr", bufs=1, space="PSUM") as scr,
            tc.tile_pool(name="qps", bufs=1, space="PSUM") as qps,
            tc.tile_pool(name="sp", bufs=2, space="PSUM") as sp,
            tc.tile_pool(name="op", bufs=2, space="PSUM") as op,
        ):
            # ---- constants ----
            ident = consts.tile([128, 128], BF16)
            nc.sync.dma_start(out=ident, in_=ident_d[:, :])
            tri = consts.tile([128, 128], BF16)
            nc.sync.dma_start(out=tri, in_=tri_d[:, :])
            maskB = consts.tile([128, 128], BF16)
            nc.sync.dma_start(out=maskB, in_=mb_d[:, :])
            m01 = consts.tile([128, NKT], F32)
            nc.sync.dma_start(out=m01, in_=m01_d[:, :])
            m01b = consts.tile([128, NKT], BF16)
            nc.sync.dma_start(out=m01b, in_=m01b_d[:, :])
            w_sb = {}
            for name, dram in (("q", wqT_d), ("k", wkT_d), ("v", wvT_d)):
                t = consts.tile([128, DC, H], BF16, tag=f"w_{name}")
                nc.sync.dma_start(out=t, in_=dram[:, :].rearrange("p (c h) -> p c h", c=DC))
                w_sb[name] = t
            b_sb = {}
            for name, dram in (("q", bq_d), ("k", bk_d), ("v", bv_d)):
                t = consts.tile([H, 1], F32, tag=f"b_{name}")
                nc.sync.dma_start(out=t, in_=dram[:, :])
                b_sb[name] = t

            KT = big.tile([128, S], BF16, tag="KT")      # K^T [h, kpos]
            QT = big.tile([128, NQ], BF16, tag="QT")     # Q^T [h, own q]
            v_t = [None] * NKT                           # [k, h | mask] tiles

            # ---- projection emission units ----
            # one unit = one (s-chunk, d-chunk) step or an eviction step;
            # attention chunks interleave these to keep the PE fed.
            kv_cur = {}

            def proj_step(sc, dc):
                if dc == 0:
                    kv_cur["kv"] = kvps.tile([128, 1024], F32, name="kvp")
                    kv_cur["q"] = qps.tile([128, 256], F32, name="qp")
                kvp, qp = kv_cur["kv"], kv_cur["q"]
                xt = xp.tile([128, 512], BF16, tag="xt")
                nc.sync.dma_start(
                    out=xt,
                    in_=xT_d[dc * 128:(dc + 1) * 128, sc * 512:(sc + 1) * 512],
                )
                st, sp_ = (dc == 0), (dc == DC - 1)
                nc.tensor.matmul(kvp[:, 0:512], w_sb["k"][:, dc, :], xt,
                                 start=st, stop=sp_)
                nc.tensor.matmul(kvp[:, 512:1024], w_sb["v"][:, dc, :], xt,
                                 start=st, stop=sp_)
                # own query tiles sit at even 128-positions: cols 0:128, 256:384
                nc.tensor.matmul(qp[:, 0:128], w_sb["q"][:, dc, :],
                                 xt[:, 0:128], start=st, stop=False)
                nc.tensor.matmul(qp[:, 128:256], w_sb["q"][:, dc, :],
                                 xt[:, 256:384], start=False,
                                 stop=sp_)

            def proj_evict(sc):
                kvp, qp = kv_cur["kv"], kv_cur["q"]
                nc.vector.tensor_scalar_add(
                    KT[:, sc * 512:(sc + 1) * 512], kvp[:, 0:512], b_sb["k"])
                vst = vstage.tile([128, 512], BF16, tag="vst")
                nc.vector.tensor_scalar_add(vst, kvp[:, 512:1024], b_sb["v"])
                nc.vector.tensor_scalar_add(
                    QT[:, sc * 256:(sc + 1) * 256], qp, b_sb["q"])
                return vst

            def proj_vt(sc, i, vst):
                pos = 4 * sc + i
                tp = scr.tile([128, 128], BF16, tag="s")
                nc.tensor.transpose(tp, vst[:, i * 128:(i + 1) * 128], ident)
                vm = vtiles.tile([128, 129], BF16, tag="v_t")
                nc.vector.tensor_scalar_mul(vm[:, 0:128], tp, m01[:, pos:pos + 1])
                nc.vector.tensor_copy(vm[:, 128:129], m01b[:, pos:pos + 1])
                v_t[pos] = vm

            def proj_units(sc):
                for dc in range(DC):
                    yield lambda dc=dc: proj_step(sc, dc)
                holder = {}

                def ev():
                    holder["vst"] = proj_evict(sc)
                yield ev
                for i in range(4):
                    yield lambda i=i: proj_vt(sc, i, holder["vst"])

            # ---- attention chunk j over owned query cols [512j, 512j+512) ----
            # q-block qi (0..3) is own tile t=4j+qi at key-position 8j+2qi;
            # PV for (qi, kt) needed for kt <= 8j+2qi+1. Edge tiles:
            # kt==8j+2qi -> triangular; kt==8j+2qi+1 -> parity mask.
            def attention(j, filler):
                n_kt = 8 * j + 8
                opsA = op.tile([128, 258], F32, tag="o2")   # qi 0,1
                opsB = op.tile([128, 258], F32, tag="o2")   # qi 2,3
                pts = [None] * n_kt
                pt2s = [None] * n_kt

                def score_exp(kt):
                    hi_only = kt > 8 * j + 3
                    w = 256 if hi_only else 512
                    qoff = j * 512 + (256 if hi_only else 0)
                    spsum = sp.tile([128, 512], F32, tag="sp")
                    nc.tensor.matmul(
                        spsum[:, 0:w], KT[:, kt * 128:(kt + 1) * 128],
                        QT[:, qoff:qoff + w], start=True, stop=True)
                    pt = ptp.tile([128, 512], BF16, tag="pt")
                    nc.scalar.activation(
                        pt[:, 0:w], spsum[:, 0:w],
                        mybir.ActivationFunctionType.Exp)
                    pts[kt] = pt
                    if kt >= 8 * j:
                        qi_e, c = (kt - 8 * j) // 2, (kt - 8 * j) % 2
                        lo = qi_e * 128 - (256 if hi_only else 0)
                        pt2 = pt2p.tile([128, 128], BF16, tag="pt2")
                        nc.vector.tensor_mul(
                            pt2, tri if c == 0 else maskB, pt[:, lo:lo + 128])
                        pt2s[kt] = pt2

                def pv(kt):
                    hi_only = kt > 8 * j + 3
                    qi_min = max(0, -(-(kt - 8 * j - 1) // 2))
                    for qi in range(qi_min, 4):
                        edge = kt >= 8 * j and (kt - 8 * j) // 2 == qi
                        if edge:
                            stat = pt2s[kt]
                        else:
                            lo = qi * 128 - (256 if hi_only else 0)
                            stat = pts[kt][:, lo:lo + 128]
                        ops = opsA if qi < 2 else opsB
                        col = (qi % 2) * 129
                        nc.tensor.matmul(
                            ops[:, col:col + 129], stat, v_t[kt][:, 0:129],
                            start=(kt == 0 and qi % 2 == 0),
                            stop=(qi == 1 and kt == 8 * j + 3)
                            or (qi == 3 and kt == n_kt - 1),
                        )

                def epilogue(qi):
                    ops = opsA if qi < 2 else opsB
                    col = (qi % 2) * 129
                    r = rp.tile([128, 1], F32, tag="r")
                    nc.vector.reciprocal(r, ops[:, col + 128:col + 129])
                    o_sb = osb.tile([128, 128], F32, tag="o")
                    nc.vector.tensor_scalar_mul(o_sb, ops[:, col:col + 128], r)
                    row = (4 * j + qi) * 128
                    nc.gpsimd.dma_start(out=o_d[row:row + 128, :], in_=o_sb)

                score_exp(0)
                for kt in range(1, n_kt):
                    score_exp(kt)
                    for f in filler.pop_units(2):
                        f()
                    pv(kt - 1)
                    if kt - 1 == 8 * j + 3:
                        epilogue(0)
                        epilogue(1)
                pv(n_kt - 1)
                epilogue(2)
                epilogue(3)

            class Filler:
                def __init__(self):
                    self.units = []

                def add(self, sc):
                    self.units.extend(proj_units(sc))

                def pop_units(self, k):
                    for _ in range(k):
                        if self.units:
                            yield self.units.pop(0)

                def drain(self):
                    yield from self.pop_units(len(self.units))

            filler = Filler()
            filler.add(0)
            filler.add(1)
            for f in filler.drain():
                f()
            for j in range(NJ):
                if 2 * j + 2 < NSC:
                    filler.add(2 * j + 2)
                    filler.add(2 * j + 3)
                attention(j, filler)
                for f in filler.drain():
                    f()
    nc.compile()
    return nc


_NC_CACHE = {}


def _get_program():
    if "nc" not in _NC_CACHE:
        _NC_CACHE["nc"] = _build_program()
    return _NC_CACHE["nc"]


def _make_in_maps(x, mask, Wq, bq, Wk, bk, Wv, bv):
    x = np.asarray(x, np.float32)
    mask = np.asarray(mask)
    scale = 1.0 / np.sqrt(np.float32(H))
    bf16 = ml_dtypes.bfloat16

    def pack_w(w):
        # [H,D] -> w.T [D,H] -> partition-major [128, DC*H] for a single
        # contiguous-burst DMA into the SBUF weight tile
        wT = np.asarray(w, np.float32).T.reshape(DC, 128, H)
        return np.ascontiguousarray(
            wT.transpose(1, 0, 2).reshape(128, DC * H).astype(bf16))

    wqT = pack_w(np.asarray(Wq, np.float32) * scale)
    wkT = pack_w(Wk)
    wvT = pack_w(Wv)
    bq_c = (np.asarray(bq, np.float32) * scale).reshape(H, 1).copy()
    bk_c = np.asarray(bk, np.float32).reshape(H, 1).copy()
    bv_c = np.asarray(bv, np.float32).reshape(H, 1).copy()
    ident = np.eye(128, dtype=bf16)
    tri = np.triu(np.ones((128, 128), np.float32)).astype(bf16)  # [k,q] keep q>=k

    in_maps = []
    for c in range(8):
        b, p = c // 2, c % 2
        # permuted tile order: even positions = own tiles (parity p)
        perm = np.arange(NKT).reshape(-1, 2)
        if p == 1:
            perm = perm[:, ::-1]
        perm = perm.reshape(-1)                                # pos -> global tile
        xT = x[b].T.reshape(D, NKT, 128)[:, perm, :].reshape(D, S)
        maskB = (np.zeros if p == 0 else np.ones)((128, 128), np.float32)
        m01 = np.where(mask[b] != 0, np.float32(1.0), np.float32(0.0))
        m01 = np.ascontiguousarray(m01.reshape(NKT, 128)[perm, :].T)
        in_maps.append({
            "xT": np.ascontiguousarray(xT.astype(bf16)),
            "wqT": wqT, "wkT": wkT, "wvT": wvT,
            "bq": bq_c, "bk": bk_c, "bv": bv_c,
            "ident": ident, "tri": tri, "maskB": maskB.astype(bf16),
            "m01": m01, "m01b": m01.astype(bf16),
        })
    return in_maps


def _install_ntff_hook():
    # the trimmed antenv package lacks axon_hooks; recreate it and wire the
    # ctypes NTFF profiling hook from trn_agent_boot so trace=True works
    import types
    if "antenv.axon_hooks" in sys.modules:
        return
    import antenv
    mod = types.ModuleType("antenv.axon_hooks")
    _hook = [None]
    mod.set_axon_ntff_profile_hook = lambda h: _hook.__setitem__(0, h)
    mod.get_axon_ntff_profile_hook = lambda: _hook[0]
    sys.modules["antenv.axon_hooks"] = mod
    antenv.axon_hooks = mod
    from trn_agent_boot.trn_boot import _ntff_profile_via_ctypes
    mod.set_axon_ntff_profile_hook(
        _ntff_profile_via_ctypes("/opt/axon/libaxon_pjrt.so"))


def run(inputs, trace=False, tmpdir=None):
    if trace:
        try:
            _install_ntff_hook()
        except Exception as e:
            print("ntff hook install failed:", e)
    nc = _get_program()
    in_maps = _make_in_maps(**inputs)
    res = bass_utils.run_bass_kernel_spmd(
        nc, in_maps, core_ids=list(range(8)), trace=trace, tmpdir=tmpdir)
    out = np.empty((B, S, H), np.float32)
    for c in range(8):
        b, p = c // 2, c % 2
        o = res.results[c]["o"]                                # [NQ, H]
        for t in range(16):
            g = 2 * t + p
            out[b, g * 128:(g + 1) * 128, :] = o[t * 128:(t + 1) * 128, :]
    return out, res


def kernel(**inputs) -> np.ndarray:
    out, _ = run(inputs, trace=False)
    return out


# revision 5
# speedup vs baseline: 1.4105x; 1.0229x over previous
"""Masked causal self-attention (single head) on 8 Trainium2 NeuronCores.

Problem: x[4,4096,1024], mask[4,4096] (key padding), Wq/Wk/Wv[128,1024],
bq/bk/bv[128] -> out[4,4096,128]:
    q = x@Wq.T+bq; k = x@Wk.T+bk; v = x@Wv.T+bv
    out = softmax(causal_mask(q@k.T/sqrt(128)) + key_padding) @ v

Sharding (SPMD, one program on 8 cores): core c = (batch b=c//2, parity
p=c%2). Each core computes K/V for its full batch and handles the
interleaved query 128-row tiles {2*t+p : t in 0..15} — interleaving
balances the causal (triangular) work between the pair.

To keep the program core-uniform, the host PERMUTES x's sequence tiles per
core so the core's own query tiles always sit at EVEN 128-column positions
(p=0: natural order; p=1: pairwise swap). All causal structure is then
position-uniform; the one residual parity difference (whether the odd
neighbor tile is a past or future key) is data (maskB2 below).

Device algorithm per core (everything on the PE runs in bf16, 1 cycle/row
at any moving size; PSUM accumulates fp32):
  - x streams in as one batched DMA per 512-column chunk ([128, 8, 512]
    d-major tile). K^T/V^T/Q^T project with the d-contraction on
    partitions, accumulating over 8 d-chunks in PSUM; Q uses the same
    resident x tiles (own tiles = even 128-blocks, one strided-AP matmul)
    so x is read exactly once. Biases fold in during PSUM->SBUF eviction;
    the 1/sqrt(128) score scale is folded into Wq/bq on the host.
  - V^T transposes back to [key, head] via 4 PE transposes sharing one
    PSUM accumulation region, then one strided DVE copy into the Vm
    buffer whose 129th column is a constant 1 (memset once).
  - Scores S^T[k,q] = KT-tile.T @ QT-chunk. The causal mask is applied
    INSIDE the score accumulation group on the PE: for the two edge key
    tiles of each query block, a matmul with stationary -1e30*triangle
    (resp. the parity mask) and identity moving adds -1e30 above the
    diagonal. exp() then runs on the scalar engine straight out of PSUM
    with the key-padding bias per partition; its only dependency is the
    PE, so no extra semaphore ops are legalized in. Softmax
    max-subtraction is skipped: scores are ~N(0,1) by construction.
  - attn@V runs with the exp tile as the STATIONARY operand and
    [V | ones] (129 cols) as the moving operand: out[q,128:129] then IS
    the softmax denominator, accumulated for free, and the output comes
    out in [q, head] orientation. Two accumulation groups share each PSUM
    bank (single start/stop per bank; PSUM pending-zero semantics make
    the second group's first touch an overwrite).
  - Normalization is reciprocal + per-partition scalar multiply.
  - Projections for chunk j+1 are emission-interleaved into attention
    chunk j so the PE has filler while the scalar engine exps.
"""

import sys

sys.path.insert(0, "/opt/trn_rl_repo")

import numpy as np
import ml_dtypes

import concourse.bass as bass
import concourse.bacc as bacc
import concourse.tile as tile
from concourse import mybir
from concourse import bass_utils

F32 = mybir.dt.float32
BF16 = mybir.dt.bfloat16
B, S, D, H = 4, 4096, 1024, 128
NQ = S // 2          # queries owned per core (2048)
DC = D // 128        # 8 d-chunks
NSC = S // 512       # 8 seq chunks of 512
NKT = S // 128       # 32 key tile positions
NJ = 4               # attention chunks of 512 owned queries
VW = 132             # Vm row pitch (129 used)


def _build_program():
    nc = bacc.Bacc("TRN2", target_bir_lowering=False)

    xT_d = nc.dram_tensor("xT", [D, S], BF16, kind="ExternalInput")
    wqT_d = nc.dram_tensor("wqT", [128, DC * H], BF16, kind="ExternalInput")
    wkT_d = nc.dram_tensor("wkT", [128, DC * H], BF16, kind="ExternalInput")
    wvT_d = nc.dram_tensor("wvT", [128, DC * H], BF16, kind="ExternalInput")
    bq_d = nc.dram_tensor("bq", [H, 1], F32, kind="ExternalInput")
    bk_d = nc.dram_tensor("bk", [H, 1], F32, kind="ExternalInput")
    bv_d = nc.dram_tensor("bv", [H, 1], F32, kind="ExternalInput")
    ident_d = nc.dram_tensor("ident", [128, 128], BF16, kind="ExternalInput")
    triM_d = nc.dram_tensor("triM", [128, 128], BF16, kind="ExternalInput")
    mb2_d = nc.dram_tensor("maskB2", [128, 128], BF16, kind="ExternalInput")
    mb_d = nc.dram_tensor("mb", [128, NKT], F32, kind="ExternalInput")
    o_d = nc.dram_tensor("o", [NQ, H], F32, kind="ExternalOutput")

    with tile.TileContext(nc) as tc:
        with (
            tc.tile_pool(name="consts", bufs=1) as consts,
            tc.tile_pool(name="big", bufs=1) as big,
            tc.tile_pool(name="xp", bufs=3) as xp,
            tc.tile_pool(name="vstage", bufs=2) as vstage,
            tc.tile_pool(name="ptp", bufs=6) as ptp,
            tc.tile_pool(name="osb", bufs=4) as osb,
            tc.tile_pool(name="rp", bufs=4) as rp,
            tc.tile_pool(name="kvps", bufs=1, space="PSUM") as kvps,
            tc.tile_pool(name="scr", bufs=1, space="PSUM") as scr,
            tc.tile_pool(name="qps", bufs=1, space="PSUM") as qps,
            tc.tile_pool(name="sp", bufs=2, space="PSUM") as sp,
            tc.tile_pool(name="op", bufs=2, space="PSUM") as op,
        ):
            # ---- weights first on the sync queue (critical path) ----
            w_sb = {}
            for name, dram in (("k", wkT_d), ("v", wvT_d), ("q", wqT_d)):
                t = consts.tile([128, DC, H], BF16, tag=f"w_{name}")
                nc.sync.dma_start(out=t, in_=dram[:, :].rearrange("p (c h) -> p c h", c=DC))
                w_sb[name] = t
            # ---- small consts on the gpsimd queue (off critical path) ----
            ident = consts.tile([128, 128], BF16)
            nc.gpsimd.dma_start(out=ident, in_=ident_d[:, :])
            triM = consts.tile([128, 128], BF16)
            nc.gpsimd.dma_start(out=triM, in_=triM_d[:, :])
            maskB2 = consts.tile([128, 128], BF16)
            nc.gpsimd.dma_start(out=maskB2, in_=mb2_d[:, :])
            mb = consts.tile([128, NKT], F32)
            nc.gpsimd.dma_start(out=mb, in_=mb_d[:, :])
            b_sb = {}
            for name, dram in (("q", bq_d), ("k", bk_d), ("v", bv_d)):
                t = consts.tile([H, 1], F32, tag=f"b_{name}")
                nc.gpsimd.dma_start(out=t, in_=dram[:, :])
                b_sb[name] = t

            KT = big.tile([128, S], BF16, tag="KT")      # K^T [h, kpos]
            QT = big.tile([128, NQ], BF16, tag="QT")     # Q^T [h, own q]
            Vm = big.tile([128, NKT, VW], BF16, tag="Vm")  # [k, h | ones]
            nc.vector.memset(Vm[:, :, 128:129], 1.0)

            # ---- projection emission units ----
            # one unit = one (s-chunk, d-chunk) step or an eviction step;
            # attention chunks interleave these to keep the PE fed.
            cur = {}

            def proj_dma(sc):
                xb = xp.tile([128, DC, 512], BF16, tag="xt")
                nc.sync.dma_start(
                    out=xb,
                    in_=xT_d[:, sc * 512:(sc + 1) * 512].rearrange(
                        "(c p) s -> p c s", p=128),
                )
                cur[sc] = xb

            def proj_step(sc, dc):
                if dc == 0:
                    cur["kv"] = kvps.tile([128, 1024], F32, name="kvp")
                    cur["q"] = qps.tile([128, 256], F32, name="qp")
                kvp, qp = cur["kv"], cur["q"]
                xt = cur[sc][:, dc, :]
                st, sp_ = (dc == 0), (dc == DC - 1)
                nc.tensor.matmul(kvp[:, 0:512], w_sb["k"][:, dc, :], xt,
                                 start=st, stop=sp_)
                nc.tensor.matmul(kvp[:, 512:1024], w_sb["v"][:, dc, :], xt,
                                 start=st, stop=sp_)
                # own query tiles sit at even 128-positions: cols 0:128, 256:384
                xq = bass.AP(tensor=xt.tensor, offset=xt.offset,
                             ap=[list(xt.ap[0]), [256, 2], [1, 128]])
                nc.tensor.matmul(qp, w_sb["q"][:, dc, :], xq,
                                 start=st, stop=sp_)

            def proj_evict(sc):
                kvp, qp = cur["kv"], cur["q"]
                nc.vector.tensor_scalar_add(
                    KT[:, sc * 512:(sc + 1) * 512], kvp[:, 0:512], b_sb["k"])
                vst = vstage.tile([128, 512], BF16, tag="vst")
                nc.vector.tensor_scalar_add(vst, kvp[:, 512:1024], b_sb["v"])
                nc.vector.tensor_scalar_add(
                    QT[:, sc * 256:(sc + 1) * 256], qp, b_sb["q"])
                cur["vst"] = vst

            def proj_vt(sc, i):
                # 4 transposes share one PSUM region (single start/stop group)
                if i == 0:
                    cur["tq"] = scr.tile([128, 512], BF16, name="tq")
                nc.tensor.matmul(
                    cur["tq"][:, i * 128:(i + 1) * 128],
                    cur["vst"][:, i * 128:(i + 1) * 128], ident,
                    is_transpose=True, start=(i == 0), stop=(i == 3))
                if i == 3:
                    nc.vector.tensor_copy(
                        Vm[:, 4 * sc:4 * sc + 4, 0:128],
                        cur["tq"].rearrange("p (a b) -> p a b", a=4))

            def proj_units(sc):
                for dc in range(DC):
                    yield lambda dc=dc: proj_step(sc, dc)
                yield lambda: proj_evict(sc)
                for i in range(4):
                    yield lambda i=i: proj_vt(sc, i)

            # ---- attention chunk j over owned query cols [512j, 512j+512) ----
            # q-block qi (0..3) is own tile t=4j+qi at key-position 8j+2qi;
            # PV for (qi, kt) needed for kt <= 8j+2qi+1. The causal edge is
            # applied pre-exp on the PE: kt==8j+2qi adds -1e30 above the
            # diagonal (triM), kt==8j+2qi+1 adds the parity mask (maskB2:
            # -1e30 everywhere for p=0 where the neighbor is a future key,
            # 0 for p=1 where it is a past key).
            def attention(j, filler):
                n_kt = 8 * j + 8
                opsA = op.tile([128, 258], F32, tag="o2")   # qi 0,1
                opsB = op.tile([128, 258], F32, tag="o2")   # qi 2,3
                pts = [None] * n_kt

                def score_exp(kt):
                    hi_only = kt > 8 * j + 3
                    w = 256 if hi_only else 512
                    qoff = j * 512 + (256 if hi_only else 0)
                    edge = kt >= 8 * j
                    spsum = sp.tile([128, 512], F32, tag="sp")
                    nc.tensor.matmul(
                        spsum[:, 0:w], KT[:, kt * 128:(kt + 1) * 128],
                        QT[:, qoff:qoff + w], start=True, stop=not edge)
                    if edge:
                        qi_e, c = (kt - 8 * j) // 2, (kt - 8 * j) % 2
                        lo = qi_e * 128 - (256 if hi_only else 0)
                        nc.tensor.matmul(
                            spsum[:, lo:lo + 128],
                            triM if c == 0 else maskB2, ident,
                            start=False, stop=True)
                    pt = ptp.tile([128, 512], BF16, tag="pt")
                    nc.scalar.activation(
                        pt[:, 0:w], spsum[:, 0:w],
                        mybir.ActivationFunctionType.Exp,
                        bias=mb[:, kt:kt + 1], scale=1.0)
                    pts[kt] = pt

                def pv(kt):
                    hi_only = kt > 8 * j + 3
                    qi_min = max(0, -(-(kt - 8 * j - 1) // 2))
                    for qi in range(qi_min, 4):
                        lo = qi * 128 - (256 if hi_only else 0)
                        ops = opsA if qi < 2 else opsB
                        col = (qi % 2) * 129
                        nc.tensor.matmul(
                            ops[:, col:col + 129], pts[kt][:, lo:lo + 128],
                            Vm[:, kt, 0:129],
                            start=(kt == 0 and qi % 2 == 0),
                            stop=(qi == 1 and kt == 8 * j + 3)
                            or (qi == 3 and kt == n_kt - 1),
                        )

                def epilogue(qi):
                    ops = opsA if qi < 2 else opsB
                    col = (qi % 2) * 129
                    r = rp.tile([128, 1], F32, tag="r")
                    nc.vector.reciprocal(r, ops[:, col + 128:col + 129])
                    o_sb = osb.tile([128, 128], F32, tag="o")
                    nc.vector.tensor_scalar_mul(o_sb, ops[:, col:col + 128], r)
                    row = (4 * j + qi) * 128
                    nc.gpsimd.dma_start(out=o_d[row:row + 128, :], in_=o_sb)

                score_exp(0)
                for kt in range(1, n_kt):
                    score_exp(kt)
                    for f in filler.pop_units(2):
                        f()
                    pv(kt - 1)
                    if kt - 1 == 8 * j + 3:
                        epilogue(0)
                        epilogue(1)
                pv(n_kt - 1)
                epilogue(2)
                epilogue(3)

            class Filler:
                def __init__(self):
                    self.units = []

                def add(self, sc):
                    self.units.extend(proj_units(sc))

                def pop_units(self, k):
                    for _ in range(k):
                        if self.units:
                            yield self.units.pop(0)

                def drain(self):
                    yield from self.pop_units(len(self.units))

            filler = Filler()
            proj_dma(0)
            proj_dma(1)
            proj_dma(2)
            filler.add(0)
            filler.add(1)
            for f in filler.drain():
                f()
            for j in range(NJ):
                if 2 * j + 2 < NSC:
                    proj_dma(2 * j + 3)
                    if 2 * j + 4 < NSC:
                        proj_dma(2 * j + 4)
                    filler.add(2 * j + 2)
                    filler.add(2 * j + 3)
                attention(j, filler)
                for f in filler.drain():
                    f()
    nc.compile()
    return nc


_NC_CACHE = {}


def _get_program():
    if "nc" not in _NC_CACHE:
        _NC_CACHE["nc"] = _build_program()
    return _NC_CACHE["nc"]


def _make_in_maps(x, mask, Wq, bq, Wk, bk, Wv, bv):
    x = np.asarray(x, np.float32)
    mask = np.asarray(mask)
    scale = 1.0 / np.sqrt(np.float32(H))
    bf16 = ml_dtypes.bfloat16
    NEG = np.float32(-1.0e30)

    def pack_w(w):
        # [H,D] -> w.T [D,H] -> partition-major [128, DC*H] for a single
        # contiguous-burst DMA into the SBUF weight tile
        wT = np.asarray(w, np.float32).T.reshape(DC, 128, H)
        return np.ascontiguousarray(
            wT.transpose(1, 0, 2).reshape(128, DC * H).astype(bf16))

    wqT = pack_w(np.asarray(Wq, np.float32) * scale)
    wkT = pack_w(Wk)
    wvT = pack_w(Wv)
    bq_c = (np.asarray(bq, np.float32) * scale).reshape(H, 1).copy()
    bk_c = np.asarray(bk, np.float32).reshape(H, 1).copy()
    bv_c = np.asarray(bv, np.float32).reshape(H, 1).copy()
    ident = np.eye(128, dtype=bf16)
    # score += triM.T[k, q']: -1e30 where q' < k (strict upper as [q', k])
    triM = (NEG * np.triu(np.ones((128, 128), np.float32), 1)).astype(bf16)

    in_maps = []
    for c in range(8):
        b, p = c // 2, c % 2
        # permuted tile order: even positions = own tiles (parity p)
        perm = np.arange(NKT).reshape(-1, 2)
        if p == 1:
            perm = perm[:, ::-1]
        perm = perm.reshape(-1)                                # pos -> global tile
        xT = x[b].T.reshape(D, NKT, 128)[:, perm, :].reshape(D, S)
        maskB2 = (np.full((128, 128), NEG) if p == 0
                  else np.zeros((128, 128), np.float32))
        mb = np.where(mask[b] != 0, np.float32(0.0), NEG)
        mb = np.ascontiguousarray(mb.reshape(NKT, 128)[perm, :].T)
        in_maps.append({
            "xT": np.ascontiguousarray(xT.astype(bf16)),
            "wqT": wqT, "wkT": wkT, "wvT": wvT,
            "bq": bq_c, "bk": bk_c, "bv": bv_c,
            "ident": ident, "triM": triM, "maskB2": maskB2.astype(bf16),
            "mb": mb,
        })
    return in_maps


def _install_ntff_hook():
    # the trimmed antenv package lacks axon_hooks; recreate it and wire the
    # ctypes NTFF profiling hook from trn_agent_boot so trace=True works
    import types
    if "antenv.axon_hooks" in sys.modules:
        return
    import antenv
    mod = types.ModuleType("antenv.axon_hooks")
    _hook = [None]
    mod.set_axon_ntff_profile_hook = lambda h: _hook.__setitem__(0, h)
    mod.get_axon_ntff_profile_hook = lambda: _hook[0]
    sys.modules["antenv.axon_hooks"] = mod
    antenv.axon_hooks = mod
    from trn_agent_boot.trn_boot import _ntff_profile_via_ctypes
    mod.set_axon_ntff_profile_hook(
        _ntff_profile_via_ctypes("/opt/axon/libaxon_pjrt.so"))


def run(inputs, trace=False, tmpdir=None):
    if trace:
        try:
            _install_ntff_hook()
        except Exception as e:
            print("ntff hook install failed:", e)
    nc = _get_program()
    in_maps = _make_in_maps(**inputs)
    res = bass_utils.run_bass_kernel_spmd(
        nc, in_maps, core_ids=list(range(8)), trace=trace, tmpdir=tmpdir)
    out = np.empty((B, S, H), np.float32)
    for c in range(8):
        b, p = c // 2, c % 2
        o = res.results[c]["o"]                                # [NQ, H]
        for t in range(16):
            g = 2 * t + p
            out[b, g * 128:(g + 1) * 128, :] = o[t * 128:(t + 1) * 128, :]
    return out, res


def kernel(**inputs) -> np.ndarray:
    out, _ = run(inputs, trace=False)
    return out


# revision 10
# speedup vs baseline: 1.4263x; 1.0111x over previous
"""Masked causal self-attention (single head) on 8 Trainium2 NeuronCores.

Problem: x[4,4096,1024], mask[4,4096] (key padding), Wq/Wk/Wv[128,1024],
bq/bk/bv[128] -> out[4,4096,128]:
    q = x@Wq.T+bq; k = x@Wk.T+bk; v = x@Wv.T+bv
    out = softmax(causal_mask(q@k.T/sqrt(128)) + key_padding) @ v

Sharding (SPMD, one program on 8 cores): core c = (batch b=c//2, parity
p=c%2). Each core computes K/V for its full batch and handles the
interleaved query 128-row tiles {2*t+p : t in 0..15} — interleaving
balances the causal (triangular) work between the pair.

To keep the program core-uniform, the host PERMUTES x's sequence tiles per
core so the core's own query tiles always sit at EVEN 128-column positions
(p=0: natural order; p=1: pairwise swap). All causal structure is then
position-uniform; the one residual parity difference (whether the odd
neighbor tile is a past or future key) is data (maskB2 below).

Device algorithm per core (everything on the PE runs in bf16, 1 cycle/row
at any moving size; PSUM accumulates fp32):
  - x streams in as one batched DMA per 512-column chunk ([128, 8, 512]
    d-major tile). K^T/V^T/Q^T project with the d-contraction on
    partitions, accumulating over 8 d-chunks in PSUM; Q uses the same
    resident x tiles (own tiles = even 128-blocks, one strided-AP matmul)
    so x is read exactly once. Biases fold in during PSUM->SBUF eviction;
    the 1/sqrt(128) score scale is folded into Wq/bq on the host.
  - V^T transposes back to [key, head] via 4 PE transposes sharing one
    PSUM accumulation region, then one strided DVE copy into the Vm
    buffer whose 129th column is a constant 1 (memset once).
  - Scores S^T[k,q] = KT-tile.T @ QT-chunk. The causal mask is applied
    INSIDE the score accumulation group on the PE: for the two edge key
    tiles of each query block, a matmul with stationary -1e30*triangle
    (resp. the parity mask) and identity moving adds -1e30 above the
    diagonal. exp() then runs on the scalar engine straight out of PSUM
    with the key-padding bias per partition; its only dependency is the
    PE, so no extra semaphore ops are legalized in. Softmax
    max-subtraction is skipped: scores are ~N(0,1) by construction.
  - attn@V runs with the exp tile as the STATIONARY operand and
    [V | ones] (129 cols) as the moving operand: out[q,128:129] then IS
    the softmax denominator, accumulated for free, and the output comes
    out in [q, head] orientation. Two accumulation groups share each PSUM
    bank (single start/stop per bank; PSUM pending-zero semantics make
    the second group's first touch an overwrite).
  - Normalization is reciprocal + per-partition scalar multiply.
  - Projections for chunk j+1 are emission-interleaved into attention
    chunk j so the PE has filler while the scalar engine exps.
"""

import sys

sys.path.insert(0, "/opt/trn_rl_repo")

import numpy as np
import ml_dtypes

import concourse.bass as bass
import concourse.bacc as bacc
import concourse.tile as tile
from concourse import mybir
from concourse import bass_utils

F32 = mybir.dt.float32
BF16 = mybir.dt.bfloat16
B, S, D, H = 4, 4096, 1024, 128
NQ = S // 2          # queries owned per core (2048)
DC = D // 128        # 8 d-chunks
NSC = S // 512       # 8 seq chunks of 512
NKT = S // 128       # 32 key tile positions
NJ = 4               # attention chunks of 512 owned queries
VW = 132             # Vm row pitch (129 used)


def _build_program():
    nc = bacc.Bacc("TRN2", target_bir_lowering=False)

    xT_d = nc.dram_tensor("xT", [D, S], BF16, kind="ExternalInput")
    wqT_d = nc.dram_tensor("wqT", [128, DC * H], BF16, kind="ExternalInput")
    wkT_d = nc.dram_tensor("wkT", [128, DC * H], BF16, kind="ExternalInput")
    wvT_d = nc.dram_tensor("wvT", [128, DC * H], BF16, kind="ExternalInput")
    bq_d = nc.dram_tensor("bq", [H, 1], F32, kind="ExternalInput")
    bk_d = nc.dram_tensor("bk", [H, 1], F32, kind="ExternalInput")
    bv_d = nc.dram_tensor("bv", [H, 1], F32, kind="ExternalInput")
    ident_d = nc.dram_tensor("ident", [128, 128], BF16, kind="ExternalInput")
    triM_d = nc.dram_tensor("triM", [128, 128], BF16, kind="ExternalInput")
    mb2_d = nc.dram_tensor("maskB2", [128, 128], BF16, kind="ExternalInput")
    mb_d = nc.dram_tensor("mb", [128, NKT], F32, kind="ExternalInput")
    o_d = nc.dram_tensor("o", [NQ, H], F32, kind="ExternalOutput")

    with tile.TileContext(nc) as tc:
        with (
            tc.tile_pool(name="consts", bufs=1) as consts,
            tc.tile_pool(name="big", bufs=1) as big,
            tc.tile_pool(name="xp", bufs=3) as xp,
            tc.tile_pool(name="vstage", bufs=2) as vstage,
            tc.tile_pool(name="ptp", bufs=6) as ptp,
            tc.tile_pool(name="osb", bufs=4) as osb,
            tc.tile_pool(name="rp", bufs=4) as rp,
            tc.tile_pool(name="kvps", bufs=1, space="PSUM") as kvps,
            tc.tile_pool(name="scr", bufs=1, space="PSUM") as scr,
            tc.tile_pool(name="qps", bufs=1, space="PSUM") as qps,
            tc.tile_pool(name="sp", bufs=2, space="PSUM") as sp,
            tc.tile_pool(name="op", bufs=2, space="PSUM") as op,
        ):
            # ---- weights on the gpsimd queue; sync carries only x tiles so
            # the first x chunk starts transferring as early as possible ----
            w_sb = {}
            for name, dram in (("k", wkT_d), ("v", wvT_d), ("q", wqT_d)):
                t = consts.tile([128, DC, H], BF16, tag=f"w_{name}")
                nc.gpsimd.dma_start(out=t, in_=dram[:, :].rearrange("p (c h) -> p c h", c=DC))
                w_sb[name] = t
            # ---- small consts on the gpsimd queue (off critical path) ----
            ident = consts.tile([128, 128], BF16)
            nc.gpsimd.dma_start(out=ident, in_=ident_d[:, :])
            triM = consts.tile([128, 128], BF16)
            nc.gpsimd.dma_start(out=triM, in_=triM_d[:, :])
            maskB2 = consts.tile([128, 128], BF16)
            nc.gpsimd.dma_start(out=maskB2, in_=mb2_d[:, :])
            mb = consts.tile([128, NKT], F32)
            nc.gpsimd.dma_start(out=mb, in_=mb_d[:, :])
            b_sb = {}
            for name, dram in (("q", bq_d), ("k", bk_d), ("v", bv_d)):
                t = consts.tile([H, 1], F32, tag=f"b_{name}")
                nc.gpsimd.dma_start(out=t, in_=dram[:, :])
                b_sb[name] = t

            KT = big.tile([128, S], BF16, tag="KT")      # K^T [h, kpos]
            QT = big.tile([128, NQ], BF16, tag="QT")     # Q^T [h, own q]
            Vm = big.tile([128, NKT, VW], BF16, tag="Vm")  # [k, h | ones]
            nc.vector.memset(Vm[:, :, 128:129], 1.0)

            # ---- projection emission units ----
            # one unit = one (s-chunk, d-chunk) step or an eviction step;
            # attention chunks interleave these to keep the PE fed.
            cur = {}

            def proj_dma(sc, split=False):
                if split:
                    # dc-pair DMAs so the first matmul can start after ~1/4
                    # of the chunk has landed
                    tiles = []
                    for h in range(4):
                        xb = xp.tile([128, 2, 512], BF16, tag=f"xs{h}",
                                     name=f"xs{h}", bufs=1)
                        nc.sync.dma_start(
                            out=xb,
                            in_=xT_d[h * 256:(h + 1) * 256,
                                     sc * 512:(sc + 1) * 512].rearrange(
                                "(c p) s -> p c s", p=128),
                        )
                        tiles.append(xb)
                    cur[sc] = lambda dc: tiles[dc // 2][:, dc % 2, :]
                else:
                    xb = xp.tile([128, DC, 512], BF16, tag="xt")
                    nc.sync.dma_start(
                        out=xb,
                        in_=xT_d[:, sc * 512:(sc + 1) * 512].rearrange(
                            "(c p) s -> p c s", p=128),
                    )
                    cur[sc] = lambda dc, xb=xb: xb[:, dc, :]

            def proj_step(sc, dc):
                if dc == 0:
                    cur["kv"] = kvps.tile([128, 1024], F32, name="kvp")
                    cur["q"] = qps.tile([128, 256], F32, name="qp")
                kvp, qp = cur["kv"], cur["q"]
                xt = cur[sc](dc)
                st, sp_ = (dc == 0), (dc == DC - 1)
                nc.tensor.matmul(kvp[:, 0:512], w_sb["k"][:, dc, :], xt,
                                 start=st, stop=sp_)
                nc.tensor.matmul(kvp[:, 512:1024], w_sb["v"][:, dc, :], xt,
                                 start=st, stop=sp_)
                # own query tiles sit at even 128-positions: cols 0:128, 256:384
                xq = bass.AP(tensor=xt.tensor, offset=xt.offset,
                             ap=[list(xt.ap[0]), [256, 2], [1, 128]])
                nc.tensor.matmul(qp, w_sb["q"][:, dc, :], xq,
                                 start=st, stop=sp_)

            def proj_evict(sc):
                kvp, qp = cur["kv"], cur["q"]
                nc.vector.tensor_scalar_add(
                    KT[:, sc * 512:(sc + 1) * 512], kvp[:, 0:512], b_sb["k"])
                vst = vstage.tile([128, 512], BF16, tag="vst")
                nc.vector.tensor_scalar_add(vst, kvp[:, 512:1024], b_sb["v"])
                nc.vector.tensor_scalar_add(
                    QT[:, sc * 256:(sc + 1) * 256], qp, b_sb["q"])
                cur["vst"] = vst

            def proj_vt(sc, i):
                # 4 transposes share one PSUM region (single start/stop group)
                if i == 0:
                    cur["tq"] = scr.tile([128, 512], BF16, name="tq")
                nc.tensor.matmul(
                    cur["tq"][:, i * 128:(i + 1) * 128],
                    cur["vst"][:, i * 128:(i + 1) * 128], ident,
                    is_transpose=True, start=(i == 0), stop=(i == 3))
                if i == 3:
                    nc.vector.tensor_copy(
                        Vm[:, 4 * sc:4 * sc + 4, 0:128],
                        cur["tq"].rearrange("p (a b) -> p a b", a=4))

            def proj_units(sc):
                for dc in range(DC):
                    yield lambda dc=dc: proj_step(sc, dc)
                yield lambda: proj_evict(sc)
                for i in range(4):
                    yield lambda i=i: proj_vt(sc, i)

            # ---- attention chunk j over owned query cols [512j, 512j+512) ----
            # q-block qi (0..3) is own tile t=4j+qi at key-position 8j+2qi;
            # PV for (qi, kt) needed for kt <= 8j+2qi+1. The causal edge is
            # applied pre-exp on the PE: kt==8j+2qi adds -1e30 above the
            # diagonal (triM), kt==8j+2qi+1 adds the parity mask (maskB2:
            # -1e30 everywhere for p=0 where the neighbor is a future key,
            # 0 for p=1 where it is a past key).
            def attention(j, filler):
                n_kt = 8 * j + 8
                opsA = op.tile([128, 258], F32, tag="o2")   # qi 0,1
                opsB = op.tile([128, 258], F32, tag="o2")   # qi 2,3
                pts = [None] * n_kt

                def score_exp(kt):
                    hi_only = kt > 8 * j + 3
                    w = 256 if hi_only else 512
                    qoff = j * 512 + (256 if hi_only else 0)
                    edge = kt >= 8 * j
                    spsum = sp.tile([128, 512], F32, tag="sp")
                    nc.tensor.matmul(
                        spsum[:, 0:w], KT[:, kt * 128:(kt + 1) * 128],
                        QT[:, qoff:qoff + w], start=True, stop=not edge)
                    if edge:
                        qi_e, c = (kt - 8 * j) // 2, (kt - 8 * j) % 2
                        lo = qi_e * 128 - (256 if hi_only else 0)
                        nc.tensor.matmul(
                            spsum[:, lo:lo + 128],
                            triM if c == 0 else maskB2, ident,
                            start=False, stop=True)
                    pt = ptp.tile([128, 512], BF16, tag="pt")
                    nc.scalar.activation(
                        pt[:, 0:w], spsum[:, 0:w],
                        mybir.ActivationFunctionType.Exp,
                        bias=mb[:, kt:kt + 1], scale=1.0)
                    pts[kt] = pt

                def pv(kt):
                    hi_only = kt > 8 * j + 3
                    qi_min = max(0, -(-(kt - 8 * j - 1) // 2))
                    for qi in range(qi_min, 4):
                        lo = qi * 128 - (256 if hi_only else 0)
                        ops = opsA if qi < 2 else opsB
                        col = (qi % 2) * 129
                        nc.tensor.matmul(
                            ops[:, col:col + 129], pts[kt][:, lo:lo + 128],
                            Vm[:, kt, 0:129],
                            start=(kt == 0 and qi % 2 == 0),
                            stop=(qi == 1 and kt == 8 * j + 3)
                            or (qi == 3 and kt == n_kt - 1),
                        )

                def epilogue(qi):
                    ops = opsA if qi < 2 else opsB
                    col = (qi % 2) * 129
                    r = rp.tile([128, 1], F32, tag="r")
                    nc.vector.reciprocal(r, ops[:, col + 128:col + 129])
                    o_sb = osb.tile([128, 128], F32, tag="o")
                    nc.vector.tensor_scalar_mul(o_sb, ops[:, col:col + 128], r)
                    row = (4 * j + qi) * 128
                    nc.gpsimd.dma_start(out=o_d[row:row + 128, :], in_=o_sb)

                score_exp(0)
                for kt in range(1, n_kt):
                    score_exp(kt)
                    for f in filler.pop_units(2):
                        f()
                    pv(kt - 1)
                    if kt - 1 == 8 * j + 3:
                        epilogue(0)
                        epilogue(1)
                pv(n_kt - 1)
                epilogue(2)
                epilogue(3)

            class Filler:
                def __init__(self):
                    self.units = []

                def add(self, sc):
                    self.units.extend(proj_units(sc))

                def pop_units(self, k):
                    for _ in range(k):
                        if self.units:
                            yield self.units.pop(0)

                def drain(self):
                    yield from self.pop_units(len(self.units))

            filler = Filler()
            proj_dma(0, split=True)
            proj_dma(1)
            proj_dma(2)
            filler.add(0)
            filler.add(1)
            for f in filler.drain():
                f()
            for j in range(NJ):
                if 2 * j + 2 < NSC:
                    proj_dma(2 * j + 3)
                    if 2 * j + 4 < NSC:
                        proj_dma(2 * j + 4)
                    filler.add(2 * j + 2)
                    filler.add(2 * j + 3)
                attention(j, filler)
                for f in filler.drain():
                    f()
    nc.compile()
    return nc


_NC_CACHE = {}


def _get_program():
    if "nc" not in _NC_CACHE:
        _NC_CACHE["nc"] = _build_program()
    return _NC_CACHE["nc"]


def _make_in_maps(x, mask, Wq, bq, Wk, bk, Wv, bv):
    x = np.asarray(x, np.float32)
    mask = np.asarray(mask)
    scale = 1.0 / np.sqrt(np.float32(H))
    bf16 = ml_dtypes.bfloat16
    NEG = np.float32(-1.0e30)

    def pack_w(w):
        # [H,D] -> w.T [D,H] -> partition-major [128, DC*H] for a single
        # contiguous-burst DMA into the SBUF weight tile
        wT = np.asarray(w, np.float32).T.reshape(DC, 128, H)
        return np.ascontiguousarray(
            wT.transpose(1, 0, 2).reshape(128, DC * H).astype(bf16))

    wqT = pack_w(np.asarray(Wq, np.float32) * scale)
    wkT = pack_w(Wk)
    wvT = pack_w(Wv)
    bq_c = (np.asarray(bq, np.float32) * scale).reshape(H, 1).copy()
    bk_c = np.asarray(bk, np.float32).reshape(H, 1).copy()
    bv_c = np.asarray(bv, np.float32).reshape(H, 1).copy()
    ident = np.eye(128, dtype=bf16)
    # score += triM.T[k, q']: -1e30 where q' < k (strict upper as [q', k])
    triM = (NEG * np.triu(np.ones((128, 128), np.float32), 1)).astype(bf16)

    in_maps = []
    for c in range(8):
        b, p = c // 2, c % 2
        # permuted tile order: even positions = own tiles (parity p)
        perm = np.arange(NKT).reshape(-1, 2)
        if p == 1:
            perm = perm[:, ::-1]
        perm = perm.reshape(-1)                                # pos -> global tile
        xT = x[b].T.reshape(D, NKT, 128)[:, perm, :].reshape(D, S)
        maskB2 = (np.full((128, 128), NEG) if p == 0
                  else np.zeros((128, 128), np.float32))
        mb = np.where(mask[b] != 0, np.float32(0.0), NEG)
        mb = np.ascontiguousarray(mb.reshape(NKT, 128)[perm, :].T)
        in_maps.append({
            "xT": np.ascontiguousarray(xT.astype(bf16)),
            "wqT": wqT, "wkT": wkT, "wvT": wvT,
            "bq": bq_c, "bk": bk_c, "bv": bv_c,
            "ident": ident, "triM": triM, "maskB2": maskB2.astype(bf16),
            "mb": mb,
        })
    return in_maps


def _install_ntff_hook():
    # the trimmed antenv package lacks axon_hooks; recreate it and wire the
    # ctypes NTFF profiling hook from trn_agent_boot so trace=True works
    import types
    if "antenv.axon_hooks" in sys.modules:
        return
    import antenv
    mod = types.ModuleType("antenv.axon_hooks")
    _hook = [None]
    mod.set_axon_ntff_profile_hook = lambda h: _hook.__setitem__(0, h)
    mod.get_axon_ntff_profile_hook = lambda: _hook[0]
    sys.modules["antenv.axon_hooks"] = mod
    antenv.axon_hooks = mod
    from trn_agent_boot.trn_boot import _ntff_profile_via_ctypes
    mod.set_axon_ntff_profile_hook(
        _ntff_profile_via_ctypes("/opt/axon/libaxon_pjrt.so"))


def run(inputs, trace=False, tmpdir=None):
    if trace:
        try:
            _install_ntff_hook()
        except Exception as e:
            print("ntff hook install failed:", e)
    nc = _get_program()
    in_maps = _make_in_maps(**inputs)
    res = bass_utils.run_bass_kernel_spmd(
        nc, in_maps, core_ids=list(range(8)), trace=trace, tmpdir=tmpdir)
    out = np.empty((B, S, H), np.float32)
    for c in range(8):
        b, p = c // 2, c % 2
        o = res.results[c]["o"]                                # [NQ, H]
        for t in range(16):
            g = 2 * t + p
            out[b, g * 128:(g + 1) * 128, :] = o[t * 128:(t + 1) * 128, :]
    return out, res


def kernel(**inputs) -> np.ndarray:
    out, _ = run(inputs, trace=False)
    return out


# revision 23
# speedup vs baseline: 1.4317x; 1.0038x over previous
"""Masked causal self-attention (single head) on 8 Trainium2 NeuronCores.

Problem: x[4,4096,1024], mask[4,4096] (key padding), Wq/Wk/Wv[128,1024],
bq/bk/bv[128] -> out[4,4096,128]:
    q = x@Wq.T+bq; k = x@Wk.T+bk; v = x@Wv.T+bv
    out = softmax(causal_mask(q@k.T/sqrt(128)) + key_padding) @ v

Sharding (SPMD, one program on 8 cores): core c = (batch b=c//2, parity
p=c%2). Each core computes K/V for its full batch and handles the
interleaved query 128-row tiles {2*t+p : t in 0..15} — interleaving
balances the causal (triangular) work between the pair.

To keep the program core-uniform, the host PERMUTES x's sequence tiles per
core so the core's own query tiles always sit at EVEN 128-column positions
(p=0: natural order; p=1: pairwise swap). All causal structure is then
position-uniform; the one residual parity difference (whether the odd
neighbor tile is a past or future key) is data (maskB2 below).

Device algorithm per core (everything on the PE runs in bf16, 1 cycle/row
at any moving size; PSUM accumulates fp32):
  - x streams in as one batched DMA per 512-column chunk ([128, 8, 512]
    d-major tile). K^T/V^T/Q^T project with the d-contraction on
    partitions, accumulating over 8 d-chunks in PSUM; Q uses the same
    resident x tiles (own tiles = even 128-blocks, one strided-AP matmul)
    so x is read exactly once. Biases fold in during PSUM->SBUF eviction;
    the 1/sqrt(128) score scale is folded into Wq/bq on the host.
  - V^T transposes back to [key, head] via 4 PE transposes sharing one
    PSUM accumulation region, then one strided DVE copy into the Vm
    buffer whose 129th column is a constant 1 (memset once).
  - Scores S^T[k,q] = KT-tile.T @ QT-chunk. The causal mask is applied
    INSIDE the score accumulation group on the PE: for the two edge key
    tiles of each query block, a matmul with stationary -1e30*triangle
    (resp. the parity mask) and identity moving adds -1e30 above the
    diagonal. exp() then runs on the scalar engine straight out of PSUM
    with the key-padding bias per partition; its only dependency is the
    PE, so no extra semaphore ops are legalized in. Softmax
    max-subtraction is skipped: scores are ~N(0,1) by construction.
  - attn@V runs with the exp tile as the STATIONARY operand and
    [V | ones] (129 cols) as the moving operand: out[q,128:129] then IS
    the softmax denominator, accumulated for free, and the output comes
    out in [q, head] orientation. Two accumulation groups share each PSUM
    bank (single start/stop per bank; PSUM pending-zero semantics make
    the second group's first touch an overwrite).
  - Normalization is reciprocal + per-partition scalar multiply.
  - Projections for chunk j+1 are emission-interleaved into attention
    chunk j so the PE has filler while the scalar engine exps.
"""

import sys

sys.path.insert(0, "/opt/trn_rl_repo")

import numpy as np
import ml_dtypes

import concourse.bass as bass
import concourse.bacc as bacc
import concourse.tile as tile
from concourse import mybir
from concourse import bass_utils

F32 = mybir.dt.float32
BF16 = mybir.dt.bfloat16
B, S, D, H = 4, 4096, 1024, 128
NQ = S // 2          # queries owned per core (2048)
DC = D // 128        # 8 d-chunks
NSC = S // 512       # 8 seq chunks of 512
NKT = S // 128       # 32 key tile positions
NJ = 4               # attention chunks of 512 owned queries
VW = 132             # Vm row pitch (129 used)


def _build_program():
    nc = bacc.Bacc("TRN2", target_bir_lowering=False)

    xT_d = nc.dram_tensor("xT", [D, S], BF16, kind="ExternalInput")
    wqT_d = nc.dram_tensor("wqT", [128, DC * H], BF16, kind="ExternalInput")
    wkT_d = nc.dram_tensor("wkT", [128, DC * H], BF16, kind="ExternalInput")
    wvT_d = nc.dram_tensor("wvT", [128, DC * H], BF16, kind="ExternalInput")
    bq_d = nc.dram_tensor("bq", [H, 1], F32, kind="ExternalInput")
    bk_d = nc.dram_tensor("bk", [H, 1], F32, kind="ExternalInput")
    bv_d = nc.dram_tensor("bv", [H, 1], F32, kind="ExternalInput")
    ident_d = nc.dram_tensor("ident", [128, 128], BF16, kind="ExternalInput")
    triM_d = nc.dram_tensor("triM", [128, 128], BF16, kind="ExternalInput")
    mb2_d = nc.dram_tensor("maskB2", [128, 128], BF16, kind="ExternalInput")
    mb_d = nc.dram_tensor("mb", [128, NKT], F32, kind="ExternalInput")
    o_d = nc.dram_tensor("o", [NQ, H], F32, kind="ExternalOutput")

    with tile.TileContext(nc) as tc:
        with (
            tc.tile_pool(name="consts", bufs=1) as consts,
            tc.tile_pool(name="big", bufs=1) as big,
            tc.tile_pool(name="xp", bufs=3) as xp,
            tc.tile_pool(name="vstage", bufs=2) as vstage,
            tc.tile_pool(name="ptp", bufs=8) as ptp,
            tc.tile_pool(name="osb", bufs=4) as osb,
            tc.tile_pool(name="rp", bufs=4) as rp,
            tc.tile_pool(name="kvps", bufs=1, space="PSUM") as kvps,
            tc.tile_pool(name="scr", bufs=1, space="PSUM") as scr,
            tc.tile_pool(name="qps", bufs=1, space="PSUM") as qps,
            tc.tile_pool(name="sp", bufs=2, space="PSUM") as sp,
            tc.tile_pool(name="op", bufs=2, space="PSUM") as op,
        ):
            # ---- weights on the gpsimd queue; sync carries only x tiles so
            # the first x chunk starts transferring as early as possible ----
            w_sb = {}
            for name, dram in (("k", wkT_d), ("v", wvT_d), ("q", wqT_d)):
                t = consts.tile([128, DC, H], BF16, tag=f"w_{name}")
                nc.gpsimd.dma_start(out=t, in_=dram[:, :].rearrange("p (c h) -> p c h", c=DC))
                w_sb[name] = t
            # ---- small consts on the gpsimd queue (off critical path) ----
            ident = consts.tile([128, 128], BF16)
            nc.gpsimd.dma_start(out=ident, in_=ident_d[:, :])
            triM = consts.tile([128, 128], BF16)
            nc.gpsimd.dma_start(out=triM, in_=triM_d[:, :])
            maskB2 = consts.tile([128, 128], BF16)
            nc.gpsimd.dma_start(out=maskB2, in_=mb2_d[:, :])
            mb = consts.tile([128, NKT], F32)
            nc.gpsimd.dma_start(out=mb, in_=mb_d[:, :])
            b_sb = {}
            for name, dram in (("q", bq_d), ("k", bk_d), ("v", bv_d)):
                t = consts.tile([H, 1], F32, tag=f"b_{name}")
                nc.gpsimd.dma_start(out=t, in_=dram[:, :])
                b_sb[name] = t

            KT = big.tile([128, S], BF16, tag="KT")      # K^T [h, kpos]
            QT = big.tile([128, NQ], BF16, tag="QT")     # Q^T [h, own q]
            Vm = big.tile([128, NKT, VW], BF16, tag="Vm")  # [k, h | ones]
            nc.vector.memset(Vm[:, :, 128:129], 1.0)

            # PE p-state warmup: the tensor engine only reaches full clock
            # after ~3us of continuous execution. Run garbage matmuls while
            # the first x chunk is still in flight so the real work starts
            # at speed. (Reads an unwritten tile; the result is discarded.)
            wsrc = big.tile([128, 512], BF16, tag="wsrc")
            nc.vector.memset(wsrc, 0.0)
            for _ in range(18):
                wdst = sp.tile([128, 512], F32, tag="sp", name="wdst")
                nc.tensor.matmul(wdst, wsrc[:, 0:128], wsrc,
                                 start=True, stop=True)

            # ---- projection emission units ----
            # one unit = one (s-chunk, d-chunk) step or an eviction step;
            # attention chunks interleave these to keep the PE fed.
            cur = {}

            def proj_dma(sc, split=False):
                if split:
                    # dc-pair DMAs so the first matmul can start after ~1/4
                    # of the chunk has landed
                    tiles = []
                    for h in range(4):
                        xb = xp.tile([128, 2, 512], BF16, tag=f"xs{h}",
                                     name=f"xs{h}", bufs=1)
                        nc.sync.dma_start(
                            out=xb,
                            in_=xT_d[h * 256:(h + 1) * 256,
                                     sc * 512:(sc + 1) * 512].rearrange(
                                "(c p) s -> p c s", p=128),
                        )
                        tiles.append(xb)
                    cur[sc] = lambda dc: tiles[dc // 2][:, dc % 2, :]
                else:
                    xb = xp.tile([128, DC, 512], BF16, tag="xt")
                    nc.sync.dma_start(
                        out=xb,
                        in_=xT_d[:, sc * 512:(sc + 1) * 512].rearrange(
                            "(c p) s -> p c s", p=128),
                    )
                    cur[sc] = lambda dc, xb=xb: xb[:, dc, :]

            def proj_step(sc, dc):
                if dc == 0:
                    cur["kv"] = kvps.tile([128, 1024], F32, name="kvp")
                    cur["q"] = qps.tile([128, 256], F32, name="qp")
                kvp, qp = cur["kv"], cur["q"]
                xt = cur[sc](dc)
                st, sp_ = (dc == 0), (dc == DC - 1)
                nc.tensor.matmul(kvp[:, 0:512], w_sb["k"][:, dc, :], xt,
                                 start=st, stop=sp_)
                nc.tensor.matmul(kvp[:, 512:1024], w_sb["v"][:, dc, :], xt,
                                 start=st, stop=sp_)
                # own query tiles sit at even 128-positions: cols 0:128, 256:384
                xq = bass.AP(tensor=xt.tensor, offset=xt.offset,
                             ap=[list(xt.ap[0]), [256, 2], [1, 128]])
                nc.tensor.matmul(qp, w_sb["q"][:, dc, :], xq,
                                 start=st, stop=sp_)

            def proj_evict(sc):
                kvp, qp = cur["kv"], cur["q"]
                nc.vector.tensor_scalar_add(
                    KT[:, sc * 512:(sc + 1) * 512], kvp[:, 0:512], b_sb["k"])
                vst = vstage.tile([128, 512], BF16, tag="vst")
                nc.vector.tensor_scalar_add(vst, kvp[:, 512:1024], b_sb["v"])
                nc.vector.tensor_scalar_add(
                    QT[:, sc * 256:(sc + 1) * 256], qp, b_sb["q"])
                cur["vst"] = vst

            def proj_vt(sc, i):
                # 4 transposes share one PSUM region (single start/stop group)
                if i == 0:
                    cur["tq"] = scr.tile([128, 512], BF16, name="tq")
                nc.tensor.matmul(
                    cur["tq"][:, i * 128:(i + 1) * 128],
                    cur["vst"][:, i * 128:(i + 1) * 128], ident,
                    is_transpose=True, start=(i == 0), stop=(i == 3))
                if i == 3:
                    nc.vector.tensor_copy(
                        Vm[:, 4 * sc:4 * sc + 4, 0:128],
                        cur["tq"].rearrange("p (a b) -> p a b", a=4))

            def proj_units(sc):
                for dc in range(DC):
                    yield lambda dc=dc: proj_step(sc, dc)
                yield lambda: proj_evict(sc)
                for i in range(4):
                    yield lambda i=i: proj_vt(sc, i)

            # ---- attention chunk j over owned query cols [512j, 512j+512) ----
            # q-block qi (0..3) is own tile t=4j+qi at key-position 8j+2qi;
            # PV for (qi, kt) needed for kt <= 8j+2qi+1. The causal edge is
            # applied pre-exp on the PE: kt==8j+2qi adds -1e30 above the
            # diagonal (triM), kt==8j+2qi+1 adds the parity mask (maskB2:
            # -1e30 everywhere for p=0 where the neighbor is a future key,
            # 0 for p=1 where it is a past key).
            def attention(j, filler):
                n_kt = 8 * j + 8
                opsA = op.tile([128, 258], F32, tag="o2")   # qi 0,1
                opsB = op.tile([128, 258], F32, tag="o2")   # qi 2,3
                pts = [None] * n_kt

                def score_exp(kt):
                    hi_only = kt > 8 * j + 3
                    w = 256 if hi_only else 512
                    qoff = j * 512 + (256 if hi_only else 0)
                    edge = kt >= 8 * j
                    spsum = sp.tile([128, 512], F32, tag="sp")
                    nc.tensor.matmul(
                        spsum[:, 0:w], KT[:, kt * 128:(kt + 1) * 128],
                        QT[:, qoff:qoff + w], start=True, stop=not edge)
                    if edge:
                        qi_e, c = (kt - 8 * j) // 2, (kt - 8 * j) % 2
                        lo = qi_e * 128 - (256 if hi_only else 0)
                        nc.tensor.matmul(
                            spsum[:, lo:lo + 128],
                            triM if c == 0 else maskB2, ident,
                            start=False, stop=True)
                    pt = ptp.tile([128, 512], BF16, tag="pt")
                    nc.scalar.activation(
                        pt[:, 0:w], spsum[:, 0:w],
                        mybir.ActivationFunctionType.Exp,
                        bias=mb[:, kt:kt + 1], scale=1.0)
                    pts[kt] = pt

                def pv(kt):
                    hi_only = kt > 8 * j + 3
                    qi_min = max(0, -(-(kt - 8 * j - 1) // 2))
                    for qi in range(qi_min, 4):
                        lo = qi * 128 - (256 if hi_only else 0)
                        ops = opsA if qi < 2 else opsB
                        col = (qi % 2) * 129
                        nc.tensor.matmul(
                            ops[:, col:col + 129], pts[kt][:, lo:lo + 128],
                            Vm[:, kt, 0:129],
                            start=(kt == 0 and qi % 2 == 0),
                            stop=(qi == 1 and kt == 8 * j + 3)
                            or (qi == 3 and kt == n_kt - 1),
                        )

                def epilogue(qi):
                    ops = opsA if qi < 2 else opsB
                    col = (qi % 2) * 129
                    r = rp.tile([128, 1], F32, tag="r")
                    nc.vector.reciprocal(r, ops[:, col + 128:col + 129])
                    o_sb = osb.tile([128, 128], F32, tag="o")
                    nc.vector.tensor_scalar_mul(o_sb, ops[:, col:col + 128], r)
                    row = (4 * j + qi) * 128
                    nc.gpsimd.dma_start(out=o_d[row:row + 128, :], in_=o_sb)

                score_exp(0)
                for kt in range(1, n_kt):
                    score_exp(kt)
                    for f in filler.pop_units(4 if j == 0 else 2):
                        f()
                    pv(kt - 1)
                    if kt - 1 == 8 * j + 3:
                        epilogue(0)
                        epilogue(1)
                pv(n_kt - 1)
                epilogue(2)
                epilogue(3)

            class Filler:
                def __init__(self):
                    self.units = []

                def add(self, sc):
                    self.units.extend(proj_units(sc))

                def pop_units(self, k):
                    for _ in range(k):
                        if self.units:
                            yield self.units.pop(0)

                def drain(self):
                    yield from self.pop_units(len(self.units))

            filler = Filler()
            proj_dma(0, split=True)
            proj_dma(1)
            proj_dma(2)
            filler.add(0)
            filler.add(1)
            for f in filler.drain():
                f()
            for j in range(NJ):
                if 2 * j + 2 < NSC:
                    proj_dma(2 * j + 3)
                    if 2 * j + 4 < NSC:
                        proj_dma(2 * j + 4)
                    filler.add(2 * j + 2)
                    filler.add(2 * j + 3)
                attention(j, filler)
                for f in filler.drain():
                    f()
    nc.compile()
    return nc


_NC_CACHE = {}


def _get_program():
    if "nc" not in _NC_CACHE:
        _NC_CACHE["nc"] = _build_program()
    return _NC_CACHE["nc"]


def _make_in_maps(x, mask, Wq, bq, Wk, bk, Wv, bv):
    x = np.asarray(x, np.float32)
    mask = np.asarray(mask)
    scale = 1.0 / np.sqrt(np.float32(H))
    bf16 = ml_dtypes.bfloat16
    NEG = np.float32(-1.0e30)

    def pack_w(w):
        # [H,D] -> w.T [D,H] -> partition-major [128, DC*H] for a single
        # contiguous-burst DMA into the SBUF weight tile
        wT = np.asarray(w, np.float32).T.reshape(DC, 128, H)
        return np.ascontiguousarray(
            wT.transpose(1, 0, 2).reshape(128, DC * H).astype(bf16))

    wqT = pack_w(np.asarray(Wq, np.float32) * scale)
    wkT = pack_w(Wk)
    wvT = pack_w(Wv)
    bq_c = (np.asarray(bq, np.float32) * scale).reshape(H, 1).copy()
    bk_c = np.asarray(bk, np.float32).reshape(H, 1).copy()
    bv_c = np.asarray(bv, np.float32).reshape(H, 1).copy()
    ident = np.eye(128, dtype=bf16)
    # score += triM.T[k, q']: -1e30 where q' < k (strict upper as [q', k])
    triM = (NEG * np.triu(np.ones((128, 128), np.float32), 1)).astype(bf16)

    in_maps = []
    for c in range(8):
        b, p = c // 2, c % 2
        # permuted tile order: even positions = own tiles (parity p)
        perm = np.arange(NKT).reshape(-1, 2)
        if p == 1:
            perm = perm[:, ::-1]
        perm = perm.reshape(-1)                                # pos -> global tile
        xT = x[b].T.reshape(D, NKT, 128)[:, perm, :].reshape(D, S)
        maskB2 = (np.full((128, 128), NEG) if p == 0
                  else np.zeros((128, 128), np.float32))
        mb = np.where(mask[b] != 0, np.float32(0.0), NEG)
        mb = np.ascontiguousarray(mb.reshape(NKT, 128)[perm, :].T)
        in_maps.append({
            "xT": np.ascontiguousarray(xT.astype(bf16)),
            "wqT": wqT, "wkT": wkT, "wvT": wvT,
            "bq": bq_c, "bk": bk_c, "bv": bv_c,
            "ident": ident, "triM": triM, "maskB2": maskB2.astype(bf16),
            "mb": mb,
        })
    return in_maps


def _install_ntff_hook():
    # the trimmed antenv package lacks axon_hooks; recreate it and wire the
    # ctypes NTFF profiling hook from trn_agent_boot so trace=True works
    import types
    if "antenv.axon_hooks" in sys.modules:
        return
    import antenv
    mod = types.ModuleType("antenv.axon_hooks")
    _hook = [None]
    mod.set_axon_ntff_profile_hook = lambda h: _hook.__setitem__(0, h)
    mod.get_axon_ntff_profile_hook = lambda: _hook[0]
    sys.modules["antenv.axon_hooks"] = mod
    antenv.axon_hooks = mod
    from trn_agent_boot.trn_boot import _ntff_profile_via_ctypes
    mod.set_axon_ntff_profile_hook(
        _ntff_profile_via_ctypes("/opt/axon/libaxon_pjrt.so"))


def run(inputs, trace=False, tmpdir=None):
    if trace:
        try:
            _install_ntff_hook()
        except Exception as e:
            print("ntff hook install failed:", e)
    nc = _get_program()
    in_maps = _make_in_maps(**inputs)
    res = bass_utils.run_bass_kernel_spmd(
        nc, in_maps, core_ids=list(range(8)), trace=trace, tmpdir=tmpdir)
    out = np.empty((B, S, H), np.float32)
    for c in range(8):
        b, p = c // 2, c % 2
        o = res.results[c]["o"]                                # [NQ, H]
        for t in range(16):
            g = 2 * t + p
            out[b, g * 128:(g + 1) * 128, :] = o[t * 128:(t + 1) * 128, :]
    return out, res


def kernel(**inputs) -> np.ndarray:
    out, _ = run(inputs, trace=False)
    return out
